# revision 1
# baseline (speedup 1.0000x reference)
"""Chunked local attention with global landmarks — Trainium2 Bass kernel.

Full (unsharded) inputs in, full output out. Internally shards across 8
NeuronCores: core i handles chunks [2i, 2i+1] of each batch (4 (b,chunk)
pairs = 2048 query tokens per core). Landmark means are computed per-core
(each 256-token segment lies inside exactly one 512-token chunk) and
replicated with a small AllGather.

Layout strategy (per core):
  - xT  [768, 2048]  host-pre-transposed slice (feature-major)
  - QT/KT computed feature-major [o, t] (moving operand = xT, stationary = W^T)
  - V computed token-major [t, o] (stationary = xT tiles, moving = W^T) with a
    fused ones-column per head -> PV matmul also produces softmax sums
  - scores computed transposed [k, q]: softmax exp is a single ACT pass
    (no max subtraction needed: |scaled scores| < 7) and PV needs no
    transposes anywhere. Key order = [512 local, 32 landmark].
  - normalization is decoupled from the PE pipeline: PV psum is raw-copied
    to SBUF and released; 1/sums broadcasts across partitions via a DRAM
    bounce and the normalize multiply lands in the transposed attention
    output, which is then the stationary operand of the output projection.
  - all matmuls run as float32r (1 cyc/row at N>=256); walrus requires every
    compute producer of an fp32r operand to round (out AP dtype float32r),
    and DMA-written tiles to bounce through a rounding engine (GPSIMD here).
"""

import os

import numpy as np

D = 768
H = 12
HD = 64
CH = 512
NLM = 32
B = 2
S = 8192
NCORES = 8
NCHUNK = S // CH           # 16
CPC = NCHUNK // NCORES     # 2 chunks per core per batch
NPAIR = B * CPC            # 4 (batch, chunk) pairs per core
TOK = NPAIR * CH           # 2048 tokens per core
JD = D // 128              # 6 feature tiles
SEG = S // NLM             # 256 tokens per landmark segment
SEG_PER_CORE = 8           # (b, seg) pairs owned per core
SCALE = float(HD) ** -0.5
NKT = 4                    # local key tiles of 128 (512 = 4*128)

_CACHE = {}


def _build():
    """Build the SPMD Bass/Tile program (same program on all 8 cores)."""
    from contextlib import ExitStack

    import concourse.bass as bass
    import concourse.tile as tile
    from concourse import bacc, mybir

    f32 = mybir.dt.float32
    f32r = mybir.dt.float32r
    Ident = mybir.ActivationFunctionType.Identity

    nc = bacc.Bacc(
        "TRN2",
        target_bir_lowering=False,
        debug=False,
        num_devices=NCORES,
    )

    xT_d = nc.dram_tensor("xT", [D, TOK], f32, kind="ExternalInput").ap()
    wqT_d = nc.dram_tensor("wqT", [D, D], f32, kind="ExternalInput").ap()
    wkT_d = nc.dram_tensor("wkT", [D, D], f32, kind="ExternalInput").ap()
    wvT_d = nc.dram_tensor("wvT", [D, D], f32, kind="ExternalInput").ap()
    woT_d = nc.dram_tensor("woT", [D, D], f32, kind="ExternalInput").ap()
    bq_d = nc.dram_tensor("bq", [D], f32, kind="ExternalInput").ap()
    bk_d = nc.dram_tensor("bk", [D], f32, kind="ExternalInput").ap()
    bv_d = nc.dram_tensor("bv", [D], f32, kind="ExternalInput").ap()
    bo_d = nc.dram_tensor("bo", [D], f32, kind="ExternalInput").ap()
    y_d = nc.dram_tensor("y", [TOK, D], f32, kind="ExternalOutput").ap()

    # landmark partial sums: [1, 128, JD, 8] -> allgather -> [8, 128, JD, 8]
    lm_part_d = nc.dram_tensor("lm_part", [1, 128, JD, SEG_PER_CORE], f32).ap()
    lm_all_d = nc.dram_tensor(
        "lm_all", [NCORES, 128, JD, SEG_PER_CORE], f32, addr_space="Shared"
    ).ap()

    def r(ap):
        return ap.bitcast(f32r)

    with tile.TileContext(nc) as tc, ExitStack() as ctx:
        wpool = ctx.enter_context(tc.tile_pool(name="w", bufs=1))
        const = ctx.enter_context(tc.tile_pool(name="const", bufs=1))
        xpool = ctx.enter_context(tc.tile_pool(name="x", bufs=2))
        qkv = ctx.enter_context(tc.tile_pool(name="qkv", bufs=1))
        ppool = ctx.enter_context(tc.tile_pool(name="p", bufs=2))
        aopool = ctx.enter_context(tc.tile_pool(name="ao", bufs=1))
        ypool = ctx.enter_context(tc.tile_pool(name="y", bufs=2))
        small = ctx.enter_context(tc.tile_pool(name="small", bufs=3))
        drpool = ctx.enter_context(tc.tile_pool(name="dr", bufs=4, space="DRAM"))
        # PSUM: 2 wide slots (2 banks each) + 4 narrow slots (1 bank) = 8 banks
        psW = ctx.enter_context(tc.tile_pool(name="psW", bufs=2, space="PSUM"))
        psN = ctx.enter_context(tc.tile_pool(name="psN", bufs=4, space="PSUM"))

        # ---- resident tiles ----
        bq_s = const.tile([128, JD], f32, tag="bq")
        bk_s = const.tile([128, JD], f32, tag="bk")
        for b_s, b_d in ((bq_s, bq_d), (bk_s, bk_d)):
            nc.sync.dma_start(out=b_s[:], in_=b_d.rearrange("(j p) -> p j", p=128))
        bqs_s = const.tile([128, JD], f32, tag="bqs")
        nc.scalar.mul(bqs_s[:], bq_s[:], SCALE)

        bv_bc = const.tile([128, D], f32, tag="bv_bc")
        bo_bc = const.tile([128, D], f32, tag="bo_bc")
        for b_s, b_d in ((bv_bc, bv_d), (bo_bc, bo_d)):
            src = bass.AP(tensor=b_d.tensor, offset=b_d.offset, ap=[[0, 128]] + list(b_d.ap))
            nc.sync.dma_start(out=b_s[:], in_=src)

        lmT_s = const.tile([128, JD, B * NLM], f32, tag="lmT")
        lmraw_s = const.tile([128, JD, B * NLM], f32, tag="lmraw")
        klmT_s = const.tile([128, JD, B * NLM], f32, tag="klmT")
        # per-batch landmark V in [tok, head, hd+1] layout, ones in col 64
        vlm_s = [
            const.tile([NLM, H, HD + 1], f32, tag=f"vlm{b}", name=f"vlm{b}")
            for b in range(B)
        ]
        lm_ps = const.tile([128, JD, SEG_PER_CORE], f32, tag="lm_ps")

        # ---- phase 1: landmark partial sums + allgather ----
        for p in range(NPAIR):
            xt = xpool.tile([128, JD, CH], f32, tag="xt")
            nc.sync.dma_start(
                out=xt[:],
                in_=xT_d[:, p * CH : (p + 1) * CH].rearrange("(j p) t -> p j t", p=128),
            )
            b, _ = divmod(p, CPC)
            off = b * 4 + 2 * (p % CPC)
            for j in range(JD):
                nc.vector.reduce_sum(
                    out=lm_ps[:, j, off : off + 2],
                    in_=xt[:, j, :].rearrange("p (s t) -> p s t", t=SEG),
                    axis=mybir.AxisListType.X,
                )
        nc.sync.dma_start(out=lm_part_d[0], in_=lm_ps[:])
        nc.gpsimd.collective_compute(
            "AllGather",
            mybir.AluOpType.bypass,
            replica_groups=[list(range(NCORES))],
            ins=[lm_part_d[:]],
            outs=[lm_all_d[:]],
        )

        # weight loads + fp32r rounding (DVE fp32 copy = 2x mode), emitted
        # after the collective trigger so the gpsimd/DMA prologue stays short
        wq_s = wpool.tile([128, JD, D], f32, tag="wq")
        wk_s = wpool.tile([128, JD, D], f32, tag="wk")
        wv_s = wpool.tile([128, JD, D], f32, tag="wv")
        wo_s = wpool.tile([128, JD, D], f32, tag="wo")
        for w_s, w_d in ((wq_s, wqT_d), (wk_s, wkT_d), (wv_s, wvT_d), (wo_s, woT_d)):
            for j in range(JD):
                wtmp = ypool.tile([128, D], f32, tag="y_s", name="wtmp")
                nc.sync.dma_start(out=wtmp[:], in_=w_d[j * 128 : (j + 1) * 128, :])
                nc.vector.tensor_copy(r(w_s[:, j, :]), wtmp[:])
        # read back gathered landmark sums: token order = b*NLM + (core*4 + s)
        for b in range(B):
            for c in range(NCORES):
                t0 = b * NLM + c * 4
                nc.sync.dma_start(
                    out=lmraw_s[:, :, t0 : t0 + 4],
                    in_=lm_all_d[c, :, :, b * 4 : (b + 1) * 4],
                )
        nc.scalar.mul(r(lmT_s[:]), lmraw_s[:], 1.0 / SEG)  # sums -> means

        # landmark K^T: [o, tok] feature-major, both batches at once
        for jo in range(JD):
            ps = psN.tile([128, CH], f32, tag="ps_n")
            for jd in range(JD):
                nc.tensor.matmul(
                    ps[:, : B * NLM],
                    r(wk_s[:, jd, jo * 128 : (jo + 1) * 128]),
                    r(lmT_s[:, jd, :]),
                    start=(jd == 0),
                    stop=(jd == JD - 1),
                )
            nc.scalar.activation(
                out=r(klmT_s[:, jo, :]),
                in_=ps[:, : B * NLM],
                func=Ident,
                bias=bk_s[:, jo : jo + 1],
                scale=1.0,
            )
        # landmark V: token-major per batch
        for b in range(B):
            pw = psW.tile([128, 2 * CH], f32, tag="ps_w")
            for jd in range(JD):
                lhsT = r(lmT_s[:, jd, b * NLM : (b + 1) * NLM])
                nc.tensor.matmul(
                    pw[:NLM, 0:512], lhsT, r(wv_s[:, jd, 0:512]),
                    start=(jd == 0), stop=(jd == JD - 1),
                )
                nc.tensor.matmul(
                    pw[:NLM, 512:768], lhsT, r(wv_s[:, jd, 512:768]),
                    start=(jd == 0), stop=(jd == JD - 1),
                )
            nc.vector.tensor_add(
                r(vlm_s[b][:, :, 0:HD]),
                pw[:NLM, 0:D].rearrange("p (h d) -> p h d", d=HD),
                bv_bc[:NLM, :].rearrange("p (h d) -> p h d", d=HD),
            )
            nc.scalar.activation(
                out=r(vlm_s[b][:, :, HD : HD + 1]),
                in_=bv_bc[:NLM, 0:H].rearrange("p (a c) -> p a c", a=H),
                func=Ident,
                scale=0.0,
                bias=1.0,
            )

        # ---- phase 2: per (batch, chunk) pair ----
        for p in range(NPAIR):
            b = p // CPC
            xt = xpool.tile([128, JD, CH], f32, tag="xt")
            nc.sync.dma_start(
                out=xt[:],
                in_=xT_d[:, p * CH : (p + 1) * CH].rearrange("(j p) t -> p j t", p=128),
            )
            xr = xpool.tile([128, JD, CH], f32, tag="xr", bufs=1)
            nc.vector.tensor_copy(r(xr[:]), xt[:])

            # Q^T and K^T projections (feature-major [o, t])
            qT = qkv.tile([128, JD, CH], f32, tag="qT")
            kT = qkv.tile([128, JD, CH], f32, tag="kT")
            for w_s, out_s, bias_s, scl in (
                (wq_s, qT, bqs_s, SCALE),
                (wk_s, kT, bk_s, 1.0),
            ):
                for jo in range(JD):
                    ps = psW.tile([128, 2 * CH], f32, tag="ps_w")
                    for jd in range(JD):
                        nc.tensor.matmul(
                            ps[:, 0:512],
                            r(w_s[:, jd, jo * 128 : (jo + 1) * 128]),
                            r(xr[:, jd, :]),
                            start=(jd == 0),
                            stop=(jd == JD - 1),
                        )
                    nc.scalar.activation(
                        out=r(out_s[:, jo, :]),
                        in_=ps[:, 0:512],
                        func=Ident,
                        bias=bias_s[:, jo : jo + 1],
                        scale=scl,
                    )

            # V projection (token-major [t, head, hd+1] with ones column)
            v_s = qkv.tile([128, NKT, H, HD + 1], f32, tag="v")
            for tt in range(NKT):
                pw = psW.tile([128, 2 * CH], f32, tag="ps_w")
                for jd in range(JD):
                    lhsT = r(xr[:, jd, tt * 128 : (tt + 1) * 128])
                    nc.tensor.matmul(
                        pw[:, 0:512], lhsT, r(wv_s[:, jd, 0:512]),
                        start=(jd == 0), stop=(jd == JD - 1),
                    )
                    nc.tensor.matmul(
                        pw[:, 512:768], lhsT, r(wv_s[:, jd, 512:768]),
                        start=(jd == 0), stop=(jd == JD - 1),
                    )
                nc.vector.tensor_add(
                    r(v_s[:, tt, :, 0:HD]),
                    pw[:, 0:D].rearrange("p (h d) -> p h d", d=HD),
                    bv_bc[:, :].rearrange("p (h d) -> p h d", d=HD),
                )
            nc.scalar.activation(
                out=r(v_s[:, :, :, HD : HD + 1]),
                in_=bv_bc[:, 0 : NKT * H].rearrange(
                    "p (a b c) -> p a b c", a=NKT, b=H
                ),
                func=Ident,
                scale=0.0,
                bias=1.0,
            )

            # attention per head; key order = [512 local, 32 landmark]
            aoT = aopool.tile([128, JD, CH], f32, tag="aoT")
            for h in range(H):
                hp = (h % 2) * 64
                jh = h // 2
                pT = ppool.tile([128, NKT + 1, CH], f32, tag="pT", bufs=2)
                # local scores in two [128, 1024] psum tiles -> 2 big exps
                for g in range(2):
                    ps = psW.tile([128, 2 * CH], f32, tag="ps_w")
                    for i in range(2):
                        kt = 2 * g + i
                        nc.tensor.matmul(
                            ps[:, i * CH : (i + 1) * CH],
                            r(kT[hp : hp + 64, jh, kt * 128 : (kt + 1) * 128]),
                            r(qT[hp : hp + 64, jh, :]),
                            start=True,
                            stop=True,
                        )
                    nc.scalar.activation(
                        out=r(pT[:, 2 * g : 2 * g + 2, :]),
                        in_=ps[:],
                        func=mybir.ActivationFunctionType.Exp,
                    )
                psl = psN.tile([128, CH], f32, tag="ps_n")
                nc.tensor.matmul(
                    psl[:NLM, :],
                    r(klmT_s[hp : hp + 64, jh, b * NLM : (b + 1) * NLM]),
                    r(qT[hp : hp + 64, jh, :]),
                    start=True,
                    stop=True,
                )
                nc.scalar.activation(
                    out=r(pT[:NLM, NKT, :]),
                    in_=psl[:NLM, :],
                    func=mybir.ActivationFunctionType.Exp,
                )

                # PV: accumulate [65, 512]; row 64 = softmax sums (ones col)
                pv = psN.tile([128, CH], f32, tag="ps_n", name="pv")
                for kt in range(NKT):
                    nc.tensor.matmul(
                        pv[: HD + 1, :],
                        r(v_s[:, kt, h, :]),
                        r(pT[:, kt, :]),
                        start=(kt == 0),
                        stop=False,
                    )
                nc.tensor.matmul(
                    pv[: HD + 1, :],
                    r(vlm_s[b][:, h, :]),
                    r(pT[:NLM, NKT, :]),
                    start=False,
                    stop=True,
                )

                # decouple: raw-copy PV out + recip, then release psum;
                # normalization happens later against the DRAM-bounced 1/sums
                stgA = small.tile([128, CH], f32, tag="stg", name="stgA", bufs=2)
                nc.vector.reciprocal(out=stgA[HD : HD + 1, :], in_=pv[HD : HD + 1, :])
                nc.vector.tensor_copy(stgA[0:HD, :], pv[0:HD, :])
                rr_d = drpool.tile([1, CH], f32, tag="rr")
                nc.sync.dma_start(out=rr_d[:], in_=stgA[HD : HD + 1, :])
                rb = small.tile([128, CH], f32, tag="rb", bufs=2)
                nc.sync.dma_start(
                    out=rb[hp : hp + 64, :],
                    in_=bass.AP(
                        tensor=rr_d.tensor,
                        offset=rr_d.offset,
                        ap=[[0, 64]] + list(rr_d[0].ap),
                    ),
                )
                if h % 2 == 0:
                    nc.vector.tensor_mul(
                        r(aoT[0:64, jh, :]), stgA[0:HD, :], rb[0:64, :]
                    )
                else:
                    stgB = small.tile([128, CH], f32, tag="stg", name="stgB", bufs=2)
                    nc.sync.dma_start(out=stgB[64:128, :], in_=stgA[0:HD, :])
                    nc.vector.tensor_mul(
                        r(aoT[64:128, jh, :]), stgB[64:128, :], rb[64:128, :]
                    )

            # output projection: stationary aoT tiles, moving W_o^T
            for tt in range(NKT):
                pw = psW.tile([128, 2 * CH], f32, tag="ps_w")
                for jd in range(JD):
                    lhsT = r(aoT[:, jd, tt * 128 : (tt + 1) * 128])
                    nc.tensor.matmul(
                        pw[:, 0:512], lhsT, r(wo_s[:, jd, 0:512]),
                        start=(jd == 0), stop=(jd == JD - 1),
                    )
                    nc.tensor.matmul(
                        pw[:, 512:768], lhsT, r(wo_s[:, jd, 512:768]),
                        start=(jd == 0), stop=(jd == JD - 1),
                    )
                y_s = ypool.tile([128, D], f32, tag="y_s")
                nc.vector.tensor_add(y_s[:], pw[:, 0:D], bo_bc[:])
                nc.sync.dma_start(
                    out=y_d[p * CH + tt * 128 : p * CH + (tt + 1) * 128, :],
                    in_=y_s[:],
                )

    nc.compile()
    return nc


def _shard_inputs(x, Wq, bq, Wk, bk, Wv, bv, Wo, bo):
    wqT = np.ascontiguousarray(Wq.T)
    wkT = np.ascontiguousarray(Wk.T)
    wvT = np.ascontiguousarray(Wv.T)
    woT = np.ascontiguousarray(Wo.T)
    in_maps = []
    for c in range(NCORES):
        blocks = []
        for b in range(B):
            for j in range(CPC):
                ch = c * CPC + j
                blocks.append(x[b, ch * CH : (ch + 1) * CH, :])
        xc = np.concatenate(blocks, axis=0)        # [TOK, D]
        xT = np.ascontiguousarray(xc.T)            # [D, TOK]
        in_maps.append(
            {
                "xT": xT,
                "wqT": wqT, "wkT": wkT, "wvT": wvT, "woT": woT,
                "bq": np.ascontiguousarray(bq),
                "bk": np.ascontiguousarray(bk),
                "bv": np.ascontiguousarray(bv),
                "bo": np.ascontiguousarray(bo),
            }
        )
    return in_maps


def _assemble(results):
    y = np.empty((B, S, D), dtype=np.float32)
    for c in range(NCORES):
        yc = results[c]["y"]
        i = 0
        for b in range(B):
            for j in range(CPC):
                ch = c * CPC + j
                y[b, ch * CH : (ch + 1) * CH, :] = yc[i * CH : (i + 1) * CH, :]
                i += 1
    return y


def kernel(x, Wq, bq, Wk, bk, Wv, bv, Wo, bo):
    from concourse.bass_utils import run_bass_kernel_spmd

    x = np.asarray(x, dtype=np.float32)
    if "nc" not in _CACHE:
        _CACHE["nc"] = _build()
    nc = _CACHE["nc"]
    in_maps = _shard_inputs(
        x,
        np.asarray(Wq), np.asarray(bq),
        np.asarray(Wk), np.asarray(bk),
        np.asarray(Wv), np.asarray(bv),
        np.asarray(Wo), np.asarray(bo),
    )
    trace = bool(int(os.environ.get("KERNEL_TRACE", "0")))
    res = run_bass_kernel_spmd(nc, in_maps, list(range(NCORES)), trace=trace)
    if trace:
        _CACHE["last_exec_time_ns"] = res.exec_time_ns
        _CACHE["last_results"] = res
    return _assemble(res.results)



# revision 16
# speedup vs baseline: 1.2208x; 1.2208x over previous
"""Chunked local attention with global landmarks — Trainium2 Bass kernel (v2).

Full (unsharded) inputs in, full output out. Core i handles chunks [2i, 2i+1]
of each batch (4 (b,chunk) pairs = 2048 query tokens per core). Landmark
partial sums are computed per-core and replicated with a small AllGather.

v2 design (vs v1 baseline):
  - all-bf16 data path (x, weights, qT/kT, v, pT, aoT): halves DMA + SBUF,
    enables FWL weight loads; PSUM stays fp32; softmax sums/recip in fp32/f32r.
  - attention matmuls 2-head packed via PSUM base-partition tiling:
    scores row-grouped (hp 0/64), PV col-grouped (out rows 0:64/64:128),
    landmark scores 4-head packed (M=32 col strips), softmax sums as
    packed M=1 ones-matmuls accumulated alongside PV.
  - normalization: sums rows gathered (SBUF-SBUF DMA) into [12, 512] ->
    one batched DVE reciprocal per pair -> selector-matmul broadcast
    (lhsT one-hot [12, 128]) -> DVE multiply. No DRAM bounce, no [1, 512]
    reciprocals.
  - software-pipelined emission: QKV_{p+1} matmul chunks are interleaved
    into pair p's attention loop as PE "cushions" that cover ACT exp
    latency; the AllGather is hidden behind QKV_0 + pair-0's local-only
    attention phase (pair 0 defers all landmark work past the collective).
"""

import os

import numpy as np

D = 768
H = 12
HD = 64
CH = 512
NLM = 32
B = 2
S = 8192
NCORES = 8
NCHUNK = S // CH           # 16
CPC = NCHUNK // NCORES     # 2 chunks per core per batch
NPAIR = B * CPC            # 4 (batch, chunk) pairs per core
TOK = NPAIR * CH           # 2048 tokens per core
JD = D // 128              # 6 feature tiles
SEG = S // NLM             # 256 tokens per landmark segment
SCALE = float(HD) ** -0.5
NKT = 4                    # local key tiles of 128
NHP = H // 2               # 6 head pairs
NG = H // 4                # 3 four-head groups

_CACHE = {}


def _build():
    from contextlib import ExitStack

    import concourse.bass as bass
    import concourse.tile as tile
    from concourse import bacc, mybir

    f32 = mybir.dt.float32
    f32r = mybir.dt.float32r
    bf16 = mybir.dt.bfloat16
    Exp = mybir.ActivationFunctionType.Exp
    ADD = mybir.AluOpType.add
    MULT = mybir.AluOpType.mult
    X = mybir.AxisListType.X

    nc = bacc.Bacc(
        "TRN2",
        target_bir_lowering=False,
        debug=False,
        num_devices=NCORES,
    )

    xT_d = nc.dram_tensor("xT", [D, TOK], bf16, kind="ExternalInput").ap()
    wqT_d = nc.dram_tensor("wqT", [D, D], bf16, kind="ExternalInput").ap()
    wkT_d = nc.dram_tensor("wkT", [D, D], bf16, kind="ExternalInput").ap()
    wvT_d = nc.dram_tensor("wvT", [D, D], bf16, kind="ExternalInput").ap()
    woT_d = nc.dram_tensor("woT", [D, D], bf16, kind="ExternalInput").ap()
    bq_d = nc.dram_tensor("bq", [D], f32, kind="ExternalInput").ap()
    bk_d = nc.dram_tensor("bk", [D], f32, kind="ExternalInput").ap()
    bv_d = nc.dram_tensor("bv", [D], f32, kind="ExternalInput").ap()
    bo_d = nc.dram_tensor("bo", [D], f32, kind="ExternalInput").ap()
    y_d = nc.dram_tensor("y", [TOK, D], f32, kind="ExternalOutput").ap()

    sel_d = nc.dram_tensor("sel", [12, NHP * 128], f32, kind="ExternalInput").ap()
    lm_part_d = nc.dram_tensor("lm_part", [1, 128, JD, 2 * NPAIR], f32).ap()
    lm_all_d = nc.dram_tensor(
        "lm_all", [NCORES, 128, JD, 2 * NPAIR], f32, addr_space="Shared"
    ).ap()

    def r(ap):
        return ap.bitcast(f32r)

    with tile.TileContext(nc) as tc, ExitStack() as ctx:
        wpool = ctx.enter_context(tc.tile_pool(name="w", bufs=1))
        const = ctx.enter_context(tc.tile_pool(name="const", bufs=1))
        xpool = ctx.enter_context(tc.tile_pool(name="x", bufs=2))
        qkpool = ctx.enter_context(tc.tile_pool(name="qk", bufs=2))
        vpool = ctx.enter_context(tc.tile_pool(name="v", bufs=2))
        aopool = ctx.enter_context(tc.tile_pool(name="ao", bufs=2))
        spool = ctx.enter_context(tc.tile_pool(name="s", bufs=4))
        ptpool = ctx.enter_context(tc.tile_pool(name="pt", bufs=4))
        lmpool = ctx.enter_context(tc.tile_pool(name="lmp", bufs=2))
        stpool = ctx.enter_context(tc.tile_pool(name="st", bufs=7))
        smpool = ctx.enter_context(tc.tile_pool(name="sm", bufs=2))
        ypool = ctx.enter_context(tc.tile_pool(name="y", bufs=2))
        psW = ctx.enter_context(tc.tile_pool(name="psW", bufs=2, space="PSUM"))
        psN = ctx.enter_context(tc.tile_pool(name="psN", bufs=4, space="PSUM"))

        # ---- constants ----
        bq_s = const.tile([128, JD], f32, tag="bq")
        bk_s = const.tile([128, JD], f32, tag="bk")
        for b_s, b_d in ((bq_s, bq_d), (bk_s, bk_d)):
            nc.sync.dma_start(out=b_s[:], in_=b_d.rearrange("(j p) -> p j", p=128))
        bv_bc = const.tile([128, D], f32, tag="bv_bc")
        bo_bc = const.tile([128, D], f32, tag="bo_bc")
        for b_s, b_d in ((bv_bc, bv_d), (bo_bc, bo_d)):
            src = bass.AP(tensor=b_d.tensor, offset=b_d.offset, ap=[[0, 128]] + list(b_d.ap))
            nc.sync.dma_start(out=b_s[:], in_=src)
        ones_bf = const.tile([128, 1], bf16, tag="ones")
        nc.vector.memset(ones_bf[:], 1.0)
        # selector for sums broadcast: sel[r, j*128+m] = 1 iff r == head(j, m)
        # (host-built constant; DVE copy provides the f32r rounding provenance)
        sel_f = const.tile([12, NHP * 128], f32, tag="self")
        sel_r = const.tile([12, NHP * 128], f32, tag="selr")
        nc.sync.dma_start(out=sel_f[:], in_=sel_d[:, :])
        nc.vector.tensor_copy(r(sel_r[:]), sel_f[:])

        # landmark tiles
        lmraw_s = const.tile([128, JD, B * NLM], f32, tag="lmraw")
        lmT_s = const.tile([128, JD, B * NLM], bf16, tag="lmT")
        klmT_s = const.tile([128, JD, B * NLM], bf16, tag="klmT")
        vlm_t = [
            const.tile([128, NG, HD], bf16, tag=f"vlm{b}", name=f"vlm{b}")
            for b in range(B)
        ]
        lm_ps = const.tile([128, JD, 2 * NPAIR], f32, tag="lm_ps")

        # ---- phase 1: landmark partial sums + allgather trigger ----
        xt_tiles = {}

        def load_xt(p):
            xt = xpool.tile([128, JD, CH], bf16, tag="xt", name=f"xt{p}")
            nc.sync.dma_start(
                out=xt[:],
                in_=xT_d[:, p * CH : (p + 1) * CH].rearrange("(j p) t -> p j t", p=128),
            )
            xt_tiles[p] = xt

        for p in (1, 2, 3, 0):
            load_xt(p)
            xt = xt_tiles[p]
            b = p // CPC
            off = b * 2 * CPC + 2 * (p % CPC)
            for j in range(JD):
                nc.vector.reduce_sum(
                    out=lm_ps[:, j, off : off + 2],
                    in_=xt[:, j, :].rearrange("p (s t) -> p s t", t=SEG),
                    axis=X,
                )
        nc.sync.dma_start(out=lm_part_d[0], in_=lm_ps[:])
        nc.gpsimd.collective_compute(
            "AllGather",
            mybir.AluOpType.bypass,
            replica_groups=[list(range(NCORES))],
            ins=[lm_part_d[:]],
            outs=[lm_all_d[:]],
        )

        # ---- weights (bf16, used directly as matmul operands) ----
        wq_s = wpool.tile([128, JD, D], bf16, tag="wq")
        wk_s = wpool.tile([128, JD, D], bf16, tag="wk")
        wv_s = wpool.tile([128, JD, D], bf16, tag="wv")
        wo_s = wpool.tile([128, JD, D], bf16, tag="wo")
        for w_s, w_d in ((wq_s, wqT_d), (wk_s, wkT_d), (wv_s, wvT_d), (wo_s, woT_d)):
            nc.sync.dma_start(out=w_s[:], in_=w_d.rearrange("(j p) o -> p j o", p=128))

        # landmark gathered sums readback (depends on collective output)
        for b in range(B):
            for c in range(NCORES):
                t0 = b * NLM + c * NPAIR
                nc.sync.dma_start(
                    out=lmraw_s[:, :, t0 : t0 + NPAIR],
                    in_=lm_all_d[c, :, :, b * NPAIR : (b + 1) * NPAIR],
                )
        nc.vector.tensor_scalar_mul(lmT_s[:], lmraw_s[:], 1.0 / SEG)

        # ---- per-pair QKV projections, sliceable into 6 cushion chunks ----
        qkv_tiles = {}

        def qkv_chunks(p):
            """Return 6 closures emitting pair p's Q/K/V projections."""
            qT = qkpool.tile([128, JD, CH], bf16, tag="qT", name=f"qT{p}")
            kT = qkpool.tile([128, JD, CH], bf16, tag="kT", name=f"kT{p}")
            v_s = vpool.tile([128, NKT, H, HD], bf16, tag="v", name=f"v{p}")
            qkv_tiles[p] = (qT, kT, v_s)
            xt = xt_tiles[p]

            def qk_pair(w_s, out_s, bias_s, scl, c):
                ps = psW.tile([128, 2 * CH], f32, tag="ps_w")
                for jo2 in range(2):
                    jo = 2 * c + jo2
                    for jd in range(JD):
                        nc.tensor.matmul(
                            ps[:, jo2 * CH : (jo2 + 1) * CH],
                            w_s[:, jd, jo * 128 : (jo + 1) * 128],
                            xt[:, jd, :],
                            start=(jd == 0),
                            stop=(jd == JD - 1),
                        )
                for jo2 in range(2):
                    jo = 2 * c + jo2
                    if scl is None:
                        nc.vector.tensor_scalar_add(
                            out_s[:, jo, :],
                            ps[:, jo2 * CH : (jo2 + 1) * CH],
                            bias_s[:, jo : jo + 1],
                        )
                    else:
                        nc.vector.tensor_scalar(
                            out_s[:, jo, :],
                            ps[:, jo2 * CH : (jo2 + 1) * CH],
                            bias_s[:, jo : jo + 1],
                            scl,
                            ADD,
                            MULT,
                        )

            def v_tt(tt):
                ps = psW.tile([128, 2 * CH], f32, tag="ps_w")
                for jd in range(JD):
                    nc.tensor.matmul(
                        ps[:, 0:CH],
                        xt[:, jd, tt * 128 : (tt + 1) * 128],
                        wv_s[:, jd, 0:CH],
                        start=(jd == 0),
                        stop=(jd == JD - 1),
                    )
                    nc.tensor.matmul(
                        ps[:, CH:D],
                        xt[:, jd, tt * 128 : (tt + 1) * 128],
                        wv_s[:, jd, CH:D],
                        start=(jd == 0),
                        stop=(jd == JD - 1),
                    )
                nc.vector.tensor_add(
                    v_s[:, tt, :, :],
                    ps[:, 0:D].rearrange("p (h d) -> p h d", d=HD),
                    bv_bc[:, :].rearrange("p (h d) -> p h d", d=HD),
                )

            return [
                lambda: qk_pair(wq_s, qT, bq_s, SCALE, 0),
                lambda: qk_pair(wq_s, qT, bq_s, SCALE, 1),
                lambda: (qk_pair(wq_s, qT, bq_s, SCALE, 2),
                         qk_pair(wk_s, kT, bk_s, None, 0)),
                lambda: qk_pair(wk_s, kT, bk_s, None, 1),
                lambda: (qk_pair(wk_s, kT, bk_s, None, 2), v_tt(0)),
                lambda: (v_tt(1), v_tt(2), v_tt(3)),
            ]

        # ---- landmark K/V projections (first use is gated on collective) ----
        def emit_lm_kv():
            psk = psN.tile([128, CH], f32, tag="ps_n", name="klm_ps")
            for jo in range(JD):
                for jd in range(JD):
                    nc.tensor.matmul(
                        psk[:, jo * 64 : jo * 64 + B * NLM],
                        wk_s[:, jd, jo * 128 : (jo + 1) * 128],
                        lmT_s[:, jd, :],
                        start=(jd == 0),
                        stop=(jd == JD - 1),
                    )
            for jo in range(JD):
                nc.vector.tensor_scalar_add(
                    klmT_s[:, jo, :],
                    psk[:, jo * 64 : jo * 64 + B * NLM],
                    bk_s[:, jo : jo + 1],
                )
            for b in range(B):
                psv = psW.tile([128, 2 * CH], f32, tag="ps_w", name="vlm_ps")
                for jd in range(JD):
                    nc.tensor.matmul(
                        psv[:NLM, 0:CH],
                        lmT_s[:, jd, b * NLM : (b + 1) * NLM],
                        wv_s[:, jd, 0:CH],
                        start=(jd == 0),
                        stop=(jd == JD - 1),
                    )
                    nc.tensor.matmul(
                        psv[:NLM, CH:D],
                        lmT_s[:, jd, b * NLM : (b + 1) * NLM],
                        wv_s[:, jd, CH:D],
                        start=(jd == 0),
                        stop=(jd == JD - 1),
                    )
                draft = ypool.tile([128, D], bf16, tag="vdraft", name=f"vlmd{b}")
                nc.vector.tensor_add(
                    draft[:NLM, :].rearrange("p (h d) -> p h d", d=HD),
                    psv[:NLM, 0:D].rearrange("p (h d) -> p h d", d=HD),
                    bv_bc[:NLM, :].rearrange("p (h d) -> p h d", d=HD),
                )
                for h in range(H):
                    g, i = h // 4, h % 4
                    nc.sync.dma_start(
                        out=vlm_t[b][32 * i : 32 * i + 32, g, :],
                        in_=draft[0:NLM, h * HD : (h + 1) * HD],
                    )

        # ---- attention + output projection for one pair ----
        ao_tiles = {}

        def emit_att(p, cushions, split_lm=False):
            b = p // CPC
            qT, kT, v_s = qkv_tiles[p]
            aoT = aopool.tile([128, JD, CH], bf16, tag="aoT", name=f"aoT{p}")
            ao_tiles[p] = aoT
            stages = []
            sums12 = smpool.tile([12, CH], f32, tag="sums12", name=f"s12_{p}")
            pt_lms = {}
            sums_ps = None

            def lm_scores_group(g):
                psg = psN.tile([128, CH], f32, tag="ps_n", name="lmsc")
                for i in range(4):
                    h = 4 * g + i
                    hp, jh = (h % 2) * 64, h // 2
                    nc.tensor.matmul(
                        psg[32 * i : 32 * i + NLM, :],
                        klmT_s[hp : hp + 64, jh, b * NLM : (b + 1) * NLM],
                        qT[hp : hp + 64, jh, :],
                        start=True,
                        stop=True,
                        tile_position=(hp, 32 * i),
                    )
                pt_lm = lmpool.tile([128, CH], bf16, tag="pt_lm", name=f"ptlm{g}")
                nc.scalar.activation(out=pt_lm[:], in_=psg[:], func=Exp)
                pt_lms[g] = pt_lm

            def sums_drain(g, sums_psum, dst):
                st = smpool.tile([128, CH], f32, tag="sums_st", name=f"sst{g}")
                nc.vector.tensor_copy(st[:], sums_psum[:])
                for i in range(4):
                    nc.sync.dma_start(
                        out=dst[4 * g + i : 4 * g + i + 1, :],
                        in_=st[32 * i : 32 * i + 1, :],
                    )

            # -- per head-pair local loop --
            for j in range(NHP):
                g, iA, iB = j // 2, (2 * j) % 4, (2 * j + 1) % 4
                hA, hB = 2 * j, 2 * j + 1
                jh = j
                if not split_lm and j % 2 == 0:
                    lm_scores_group(g)
                # scores: head A (rows 0:64, row-grp 0) and B (64:128, grp 64)
                score_s = [
                    spool.tile([128, NKT * CH], bf16, tag="score", name=f"sc{h2}")
                    for h2 in range(2)
                ]
                for half in range(2):
                    psA = psW.tile([128, 2 * CH], f32, tag="ps_w")
                    psB = psW.tile([128, 2 * CH], f32, tag="ps_w")
                    for i2 in range(2):
                        kt = 2 * half + i2
                        for hx, psx in ((0, psA), (64, psB)):
                            nc.tensor.matmul(
                                psx[:, i2 * CH : (i2 + 1) * CH],
                                kT[hx : hx + 64, jh, kt * 128 : (kt + 1) * 128],
                                qT[hx : hx + 64, jh, :],
                                start=True,
                                stop=True,
                            )
                    nc.vector.tensor_copy(
                        score_s[0][:, half * 2 * CH : (half + 1) * 2 * CH], psA[:]
                    )
                    nc.vector.tensor_copy(
                        score_s[1][:, half * 2 * CH : (half + 1) * 2 * CH], psB[:]
                    )
                pts = []
                for h2 in range(2):
                    pt = ptpool.tile([128, NKT, CH], bf16, tag="pt", name=f"pt{h2}")
                    nc.scalar.activation(
                        out=pt[:],
                        in_=score_s[h2][:].rearrange("p (k t) -> p k t", t=CH),
                        func=Exp,
                    )
                    pts.append(pt)

                # PE cushion: next pair's QKV (or prev pair's O) chunk
                if cushions is not None:
                    cushions[j]()

                # PV + sums (2-head packed; M=1 ones-matmul sums, 4-head bank)
                pv = psN.tile([128, CH], f32, tag="ps_n", name="pv")
                if j % 2 == 0:
                    sums_ps = psN.tile([128, CH], f32, tag="ps_n", name="sums")
                for kt in range(NKT):
                    st_f = kt == 0
                    sp_f = split_lm and kt == NKT - 1
                    nc.tensor.matmul(
                        pv[0:64, :], v_s[:, kt, hA, :], pts[0][:, kt, :],
                        start=st_f, stop=sp_f,
                    )
                    nc.tensor.matmul(
                        pv[64:128, :], v_s[:, kt, hB, :], pts[1][:, kt, :],
                        start=st_f, stop=sp_f,
                    )
                    nc.tensor.matmul(
                        sums_ps[32 * iA : 32 * iA + 1, :],
                        ones_bf[:, 0:1], pts[0][:, kt, :],
                        start=st_f, stop=sp_f,
                        tile_position=(0, 32 * iA),
                    )
                    nc.tensor.matmul(
                        sums_ps[32 * iB : 32 * iB + 1, :],
                        ones_bf[:, 0:1], pts[1][:, kt, :],
                        start=st_f, stop=sp_f,
                        tile_position=(0, 32 * iB),
                    )
                if not split_lm:
                    pt_lm = pt_lms[g]
                    nc.tensor.matmul(
                        pv[0:64, :],
                        vlm_t[b][32 * iA : 32 * iA + 32, g, :],
                        pt_lm[32 * iA : 32 * iA + 32, :],
                        start=False, stop=True,
                        tile_position=(32 * iA, 0),
                    )
                    nc.tensor.matmul(
                        pv[64:128, :],
                        vlm_t[b][32 * iB : 32 * iB + 32, g, :],
                        pt_lm[32 * iB : 32 * iB + 32, :],
                        start=False, stop=True,
                        tile_position=(32 * iB, 64),
                    )
                    nc.tensor.matmul(
                        sums_ps[32 * iA : 32 * iA + 1, :],
                        ones_bf[32 * iA : 32 * iA + 32, 0:1],
                        pt_lm[32 * iA : 32 * iA + 32, :],
                        start=False, stop=True,
                        tile_position=(32 * iA, 32 * iA),
                    )
                    nc.tensor.matmul(
                        sums_ps[32 * iB : 32 * iB + 1, :],
                        ones_bf[32 * iB : 32 * iB + 32, 0:1],
                        pt_lm[32 * iB : 32 * iB + 32, :],
                        start=False, stop=True,
                        tile_position=(32 * iB, 32 * iB),
                    )
                stage = stpool.tile([128, CH], f32, tag="stage", name=f"stg{j}")
                nc.vector.tensor_copy(stage[:], pv[:])
                stages.append(stage)
                if j % 2 == 1:
                    sums_drain(g, sums_ps, sums12)

            # -- deferred landmark phase (pair 0: gated on the collective) --
            if split_lm:
                if p == 0:
                    emit_lm_kv()
                sums12b = smpool.tile([12, CH], f32, tag="s12b", name="s12b")
                for g in range(NG):
                    lm_scores_group(g)
                sums_lm = None
                for j in range(NHP):
                    g, iA, iB = j // 2, (2 * j) % 4, (2 * j + 1) % 4
                    pt_lm = pt_lms[g]
                    pv_lm = psN.tile([128, CH], f32, tag="ps_n", name="pvlm")
                    nc.tensor.matmul(
                        pv_lm[0:64, :],
                        vlm_t[b][32 * iA : 32 * iA + 32, g, :],
                        pt_lm[32 * iA : 32 * iA + 32, :],
                        start=True, stop=True,
                        tile_position=(32 * iA, 0),
                    )
                    nc.tensor.matmul(
                        pv_lm[64:128, :],
                        vlm_t[b][32 * iB : 32 * iB + 32, g, :],
                        pt_lm[32 * iB : 32 * iB + 32, :],
                        start=True, stop=True,
                        tile_position=(32 * iB, 64),
                    )
                    if j % 2 == 0:
                        sums_lm = psN.tile([128, CH], f32, tag="ps_n", name="slm")
                    nc.tensor.matmul(
                        sums_lm[32 * iA : 32 * iA + 1, :],
                        ones_bf[32 * iA : 32 * iA + 32, 0:1],
                        pt_lm[32 * iA : 32 * iA + 32, :],
                        start=True, stop=True,
                        tile_position=(32 * iA, 32 * iA),
                    )
                    nc.tensor.matmul(
                        sums_lm[32 * iB : 32 * iB + 1, :],
                        ones_bf[32 * iB : 32 * iB + 32, 0:1],
                        pt_lm[32 * iB : 32 * iB + 32, :],
                        start=True, stop=True,
                        tile_position=(32 * iB, 32 * iB),
                    )
                    nc.vector.tensor_add(stages[j][:], stages[j][:], pv_lm[:])
                    if j % 2 == 1:
                        sums_drain(g, sums_lm, sums12b)
                nc.vector.tensor_add(sums12[:], sums12[:], sums12b[:])

            # -- normalize: batched reciprocal + selector-matmul broadcast --
            recip_f = smpool.tile([12, CH], f32, tag="recipf", name=f"rcf{p}")
            recip = smpool.tile([12, CH], f32, tag="recip", name=f"rc{p}")
            nc.vector.reciprocal(out=recip_f[:], in_=sums12[:])
            nc.vector.tensor_copy(r(recip[:]), recip_f[:])
            for j in range(NHP):
                bc = psN.tile([128, CH], f32, tag="ps_n", name="bc")
                nc.tensor.matmul(
                    bc[:, :],
                    r(sel_r[:, j * 128 : (j + 1) * 128]),
                    r(recip[:]),
                    start=True,
                    stop=True,
                )
                nc.vector.tensor_mul(
                    aoT[0:64, j, :], stages[j][0:64, :], bc[0:64, :]
                )
                nc.vector.tensor_mul(
                    aoT[64:128, j, :], stages[j][64:128, :], bc[64:128, :]
                )

        def emit_o(p, as_chunks=False):
            aoT = ao_tiles[p]

            def one_tt(tt):
                pw = psW.tile([128, 2 * CH], f32, tag="ps_w")
                for jd in range(JD):
                    nc.tensor.matmul(
                        pw[:, 0:CH],
                        aoT[:, jd, tt * 128 : (tt + 1) * 128],
                        wo_s[:, jd, 0:CH],
                        start=(jd == 0),
                        stop=(jd == JD - 1),
                    )
                    nc.tensor.matmul(
                        pw[:, CH:D],
                        aoT[:, jd, tt * 128 : (tt + 1) * 128],
                        wo_s[:, jd, CH:D],
                        start=(jd == 0),
                        stop=(jd == JD - 1),
                    )
                y_s = ypool.tile([128, D], f32, tag="y_s")
                nc.vector.tensor_add(y_s[:], pw[:, 0:D], bo_bc[:])
                nc.sync.dma_start(
                    out=y_d[p * CH + tt * 128 : p * CH + (tt + 1) * 128, :],
                    in_=y_s[:],
                )

            if as_chunks:
                return [lambda: one_tt(0), lambda: one_tt(1), lambda: one_tt(2),
                        lambda: one_tt(3), lambda: None, lambda: None]
            for tt in range(NKT):
                one_tt(tt)

        # ---- main emission ----
        # prologue QKV for pair 0 (emitted inline, covers the collective)
        for chunk in qkv_chunks(0):
            chunk()

        load_xt(1)
        emit_att(0, cushions=qkv_chunks(1), split_lm=True)
        emit_o(0)

        load_xt(2)
        emit_att(1, cushions=qkv_chunks(2))
        emit_o(1)

        load_xt(3)
        emit_att(2, cushions=qkv_chunks(3))
        # O-projection of pair 2 is deferred: it becomes pair 3's cushions.

        emit_att(3, cushions=emit_o(2, as_chunks=True))
        emit_o(3)

    nc.compile()
    return nc


def _shard_inputs(x, Wq, bq, Wk, bk, Wv, bv, Wo, bo):
    import ml_dtypes

    bf = ml_dtypes.bfloat16
    wqT = np.ascontiguousarray(Wq.T).astype(bf)
    wkT = np.ascontiguousarray(Wk.T).astype(bf)
    wvT = np.ascontiguousarray(Wv.T).astype(bf)
    woT = np.ascontiguousarray(Wo.T).astype(bf)
    in_maps = []
    sel = np.zeros((12, NHP * 128), dtype=np.float32)
    for j in range(NHP):
        sel[2 * j, j * 128 : j * 128 + 64] = 1.0
        sel[2 * j + 1, j * 128 + 64 : j * 128 + 128] = 1.0
    for c in range(NCORES):
        blocks = []
        for b in range(B):
            for j in range(CPC):
                ch = c * CPC + j
                blocks.append(x[b, ch * CH : (ch + 1) * CH, :])
        xc = np.concatenate(blocks, axis=0)        # [TOK, D]
        xT = np.ascontiguousarray(xc.T).astype(bf)  # [D, TOK]
        in_maps.append(
            {
                "xT": xT,
                "sel": sel,
                "wqT": wqT, "wkT": wkT, "wvT": wvT, "woT": woT,
                "bq": np.ascontiguousarray(bq, dtype=np.float32),
                "bk": np.ascontiguousarray(bk, dtype=np.float32),
                "bv": np.ascontiguousarray(bv, dtype=np.float32),
                "bo": np.ascontiguousarray(bo, dtype=np.float32),
            }
        )
    return in_maps


def _assemble(results):
    y = np.empty((B, S, D), dtype=np.float32)
    for c in range(NCORES):
        yc = results[c]["y"]
        i = 0
        for b in range(B):
            for j in range(CPC):
                ch = c * CPC + j
                y[b, ch * CH : (ch + 1) * CH, :] = yc[i * CH : (i + 1) * CH, :]
                i += 1
    return y


def kernel(x, Wq, bq, Wk, bk, Wv, bv, Wo, bo):
    from concourse.bass_utils import run_bass_kernel_spmd

    x = np.asarray(x, dtype=np.float32)
    if "nc" not in _CACHE:
        _CACHE["nc"] = _build()
    nc = _CACHE["nc"]
    in_maps = _shard_inputs(
        x,
        np.asarray(Wq), np.asarray(bq),
        np.asarray(Wk), np.asarray(bk),
        np.asarray(Wv), np.asarray(bv),
        np.asarray(Wo), np.asarray(bo),
    )
    trace = bool(int(os.environ.get("KERNEL_TRACE", "0")))
    res = run_bass_kernel_spmd(nc, in_maps, list(range(NCORES)), trace=trace)
    if trace:
        _CACHE["last_exec_time_ns"] = res.exec_time_ns
        _CACHE["last_results"] = res
    return _assemble(res.results)
